# revision 3
# baseline (speedup 1.0000x reference)
"""Sorted-window + two-hot gather kernel.

Host globally sorts edges by src and shards contiguous ranges to cores.
Per 4096-edge tile the src values span <256 nodes, so the src gather is a
windowed one-hot (PE broadcast -> ACT copy -> 2 DVE compares -> 2 PE
accumulating matmuls). The dst side is random over all 50K nodes and is
split between the Pool 16-candidate indirect_copy route (NPOOL tiles) and
a factorized "two-hot" gather (NTH tiles): dst = hi*391 + lo, one-hot of
lo via 4 DVE compares feeding 4 PE matmuls against [128,128] slices of the
g-table, then a hi partition mask and a ones-reduce. All routes accumulate
s = g[src]+g[dst] in one PSUM tile; ACT computes the gumbel-sigmoid gate.
Output is returned in sorted order and unpermuted on the host.
"""
import sys
sys.path.insert(0, '/opt/trn_rl_repo')
import numpy as np

N, IN_DIM, HID = 50000, 256, 64
E = 1_600_000
BIAS = 0.0001
NCORES = 8
EC = E // NCORES            # 200000
TILE = 4096
NTIL = 49
ECP = NTIL * TILE           # 200704
NNC = N // NCORES           # 6250
NNCP = 6272
C2 = 391                    # two-hot stride: dst = hi*391 + lo, hi < 128
GPAD = 50432                # padded g table: 384 + 128*391 = 50432
TABW = GPAD // 16           # 3152
NTH = 16                    # two-hot dst tiles
NPOOL = NTIL - NTH          # 35 pool dst tiles
TH_SET = sorted({int((i + 0.5) * NTIL / NTH) for i in range(NTH)})
assert len(TH_SET) == NTH

_nc = None
NREP = 1
SIMM = False


def _build():
    from concourse import bass, bacc, tile, mybir

    f32 = mybir.dt.float32
    bf16 = mybir.dt.bfloat16
    u16 = mybir.dt.uint16
    f16 = mybir.dt.float16
    ACT = mybir.ActivationFunctionType
    OP = mybir.AluOpType
    nc = bacc.Bacc("TRN2", target_bir_lowering=False, debug=False,
                   num_devices=(1 if SIMM else NCORES))

    embT_d = nc.dram_tensor("embT", [2, 128, NNCP], f32, kind="ExternalInput")
    wemb_d = nc.dram_tensor("wemb", [2, 128, HID], f32, kind="ExternalInput")
    bemb_d = nc.dram_tensor("bemb", [HID, 1], f32, kind="ExternalInput")
    wbar_d = nc.dram_tensor("wbar", [HID, 1], f32, kind="ExternalInput")
    bE_d = nc.dram_tensor("bE", [8, 1], f32, kind="ExternalInput")
    gc_d = nc.dram_tensor("gc", [16, 2], f32, kind="ExternalInput")
    tsel_d = nc.dram_tensor("tsel", [16, 8], f32, kind="ExternalInput")
    cst_d = nc.dram_tensor("cst", [128, 7], f32, kind="ExternalInput")
    ones1_d = nc.dram_tensor("ones1", [1, 128], bf16, kind="ExternalInput")
    ones128_d = nc.dram_tensor("ones128", [128, 1], bf16, kind="ExternalInput")
    expand8_d = nc.dram_tensor("expand8", [8, 128], bf16, kind="ExternalInput")
    bdiag8_d = nc.dram_tensor("bdiag8", [128, 8], bf16, kind="ExternalInput")
    osel_d = nc.dram_tensor("osel", [8, 128, 8], f16, kind="ExternalInput")
    usel_d = nc.dram_tensor("usel", [8, 8, 128], f16, kind="ExternalInput")
    rel_d = nc.dram_tensor("rel", [NTIL, 8, 512], bf16, kind="ExternalInput")
    nz_d = nc.dram_tensor("nz", [NTIL, 8, 512], f32, kind="ExternalInput")
    gwidx_d = nc.dram_tensor("gwidx", [128, 16], u16, kind="ExternalInput")
    icidx_d = nc.dram_tensor("icidx", [max(NPOOL, 1), 128, 32], u16,
                             kind="ExternalInput")
    ecf_d = nc.dram_tensor("ecf", [max(NPOOL, 1), 8, 512], bf16,
                           kind="ExternalInput")
    lo_d = nc.dram_tensor("lo", [max(NTH, 1), 8, 512], f16,
                          kind="ExternalInput")
    hi_d = nc.dram_tensor("hi", [max(NTH, 1), 8, 512], f16,
                          kind="ExternalInput")
    out_d = nc.dram_tensor("out", [NTIL, 8, 512], f32, kind="ExternalOutput")

    a1, b1 = 2.0 * BIAS - 1.0, 1.0 - BIAS
    a2, b2 = 1.0 - 2.0 * BIAS, BIAS

    with tile.TileContext(nc) as tc:
        with tc.tile_pool(name="const", bufs=1) as cp, \
             tc.tile_pool(name="tab", bufs=1) as tabp, \
             tc.tile_pool(name="dram", bufs=1, space="DRAM") as dram:
            def cload(name, shape, dt, src):
                t = cp.tile(shape, dt, name=name, tag=name)
                nc.sync.dma_start(out=t[:], in_=src)
                return t
            w0 = cload("w0", [128, HID], f32, wemb_d[0])
            w1 = cload("w1", [128, HID], f32, wemb_d[1])
            bemb = cload("bemb", [HID, 1], f32, bemb_d[:, :])
            wbar = cload("wbar", [HID, 1], f32, wbar_d[:, :])
            bE = cload("bE", [8, 1], f32, bE_d[:, :])
            cst = cload("cst", [128, 7], f32, cst_d[:, :])
            ones1 = cload("ones1", [1, 128], bf16, ones1_d[:, :])
            ones128 = cload("ones128", [128, 1], bf16, ones128_d[:, :])
            expand8 = cload("expand8", [8, 128], bf16, expand8_d[:, :])
            bdiag8 = cload("bdiag8", [128, 8], bf16, bdiag8_d[:, :])
            osel = [cload(f"osel{u}", [128, 8], f16, osel_d[u])
                    for u in range(8)]
            usel = [cload(f"usel{u}", [8, 128], f16, usel_d[u])
                    for u in range(8)]
            gc = cload("gc", [16, 2], f32, gc_d[:, :])
            tsel = cload("tsel", [16, 8], f32, tsel_d[:, :])
            # bulk per-tile index/mask inputs, u-major in SBUF
            lo_all = cp.tile([8, max(NTH, 1), 512], f16, tag="lo_all")
            nc.sync.dma_start(out=lo_all[:],
                              in_=lo_d[:, :, :].rearrange("t u m -> u t m"))
            hi_all = cp.tile([8, max(NTH, 1), 512], f16, tag="hi_all")
            nc.sync.dma_start(out=hi_all[:],
                              in_=hi_d[:, :, :].rearrange("t u m -> u t m"))
            icidx_all = cp.tile([128, max(NPOOL, 1), 32], u16, tag="icidx_all")
            nc.sync.dma_start(out=icidx_all[:],
                              in_=icidx_d[:, :, :].rearrange("t p w -> p t w"))
            gwidx = cp.tile([128, 16], u16, tag="gwidx")
            nc.sync.dma_start(out=gwidx[:], in_=gwidx_d[:, :])

            # ---------- phase 1: per-node scalar g ----------
            g_sb = cp.tile([1, NNCP], f32, tag="gsb")
            with tc.tile_pool(name="p1", bufs=3) as p1, \
                 tc.tile_pool(name="ps1", bufs=2, space="PSUM") as ps1, \
                 tc.tile_pool(name="ps1g", bufs=2, space="PSUM") as ps1g:
                col = 0
                while col < NNCP:
                    n = min(512, NNCP - col)
                    r0 = p1.tile([128, n], f32, tag="r0")
                    r1 = p1.tile([128, n], f32, tag="r1")
                    nc.sync.dma_start(out=r0[:], in_=embT_d[0, :, col:col + n])
                    nc.sync.dma_start(out=r1[:], in_=embT_d[1, :, col:col + n])
                    ph = ps1.tile([HID, n], f32, tag="ph")
                    nc.tensor.matmul(out=ph[:], lhsT=w0[:], rhs=r0[:],
                                     start=True, stop=False)
                    nc.tensor.matmul(out=ph[:], lhsT=w1[:], rhs=r1[:],
                                     start=False, stop=True)
                    hT = p1.tile([HID, n], f32, tag="hT")
                    nc.scalar.activation(out=hT[:], in_=ph[:], func=ACT.Relu,
                                         bias=bemb[:, 0:1])
                    pg = ps1g.tile([1, n], f32, tag="pg")
                    nc.tensor.matmul(out=pg[:], lhsT=wbar[:], rhs=hT[:],
                                     start=True, stop=True)
                    nc.scalar.activation(out=g_sb[0:1, col:col + n], in_=pg[:],
                                         func=ACT.Identity)
                    col += n

            g_mine = dram.tile([1, NNC], f32, tag="gmine")
            g_all = dram.tile([1, GPAD], f32, tag="gall")
            nc.sync.dma_start(out=g_mine[:], in_=g_sb[0:1, 0:NNC])
            zpad = cp.tile([1, GPAD - N], f32, tag="zpad")
            nc.vector.memset(zpad[:], 0.0)
            nc.sync.dma_start(out=g_all[0:1, N:GPAD], in_=zpad[:])
            if SIMM:
                nc.sync.dma_start(out=g_all[0:1, 0:NNC], in_=g_mine[:])
            else:
                nc.gpsimd.collective_compute(
                    "AllGather", bass.mybir.AluOpType.bypass,
                    replica_groups=[list(range(NCORES))],
                    ins=[g_mine[:].opt()], outs=[g_all[0:1, 0:N].opt()])

            # ---------- tables ----------
            candtab = tabp.tile([128, TABW], f32, tag="candtab")
            g_wc = g_all[0, 0:GPAD].rearrange("(w c) -> c w", c=16)
            dma_engs = [nc.sync, nc.scalar]
            for G in range(8):
                dma_engs[G % 2].dma_start(
                    out=candtab[16 * G:16 * G + 16, :], in_=g_wc)
            TT = []
            for k in range(4):
                tmp = tabp.tile([128, 128], f32, name=f"ttf{k}", tag=f"ttf{k}")
                src_ap = g_all[0, 128 * k:128 * k + 128 * C2].rearrange(
                    "(i c) -> c i", c=C2)
                nc.sync.dma_start(out=tmp[:], in_=src_ap[0:128, :])
                ttk = tabp.tile([128, 128], f16, name=f"tt{k}", tag=f"tt{k}")
                nc.vector.tensor_copy(out=ttk[:], in_=tmp[:])
                TT.append(ttk)

            # batched per-band window fetch for all tiles:
            # gwa[16g+q, 4t+blk] = g[16*bg16[t,g] + 16*blk + q]
            gwa = tabp.tile([128, 208], f32, tag="gwa")
            nc.gpsimd.indirect_copy(
                out=gwa[:], data=candtab[:], idxs=gwidx[:, :],
                i_know_ap_gather_is_preferred=True)

            # ---------- phase 2 ----------
            pool_ord = [t for t in range(NTIL) if t not in TH_SET]
            th_ord = TH_SET
            pool_pos = {t: i for i, t in enumerate(pool_ord)}
            th_pos = {t: i for i, t in enumerate(th_ord)}

            import contextlib
            rep_ctx = tc.For_i(0, NREP) if NREP > 1 else contextlib.nullcontext()
            with rep_ctx, \
                 tc.tile_pool(name="gw", bufs=3) as gwp, \
                 tc.tile_pool(name="bc", bufs=2, space="PSUM") as bcp, \
                 tc.tile_pool(name="s2", bufs=2, space="PSUM") as s2p, \
                 tc.tile_pool(name="bh", bufs=2, space="PSUM") as bhp, \
                 tc.tile_pool(name="pss", bufs=2, space="PSUM") as pssp, \
                 tc.tile_pool(name="mk", bufs=3) as mkp, \
                 tc.tile_pool(name="cnd", bufs=2) as cndp, \
                 tc.tile_pool(name="gate", bufs=2) as gatep, \
                 tc.tile_pool(name="g2", bufs=4) as g2p:
                pend = []
                GGRP = 3
                for t in range(NTIL):
                    ps_s = pssp.tile([8, 512], f32, tag="ps_s")
                    # src: band-windowed one-hot, whole tile at once
                    rel8 = gatep.tile([8, 512], bf16, tag="rel8")
                    nc.sync.dma_start(out=rel8[:], in_=rel_d[t])
                    psB = bcp.tile([128, 512], f32, tag="bc")
                    nc.tensor.matmul(out=psB[:], lhsT=expand8[:],
                                     rhs=rel8[:], start=True, stop=True)
                    relb = mkp.tile([128, 512], bf16, tag="relb")
                    nc.scalar.activation(out=relb[:], in_=psB[:],
                                         func=ACT.Identity)
                    for blk in range(3):
                        mw = mkp.tile([128, 512], bf16, name=f"mw{blk}",
                                      tag=f"mw{blk}")
                        nc.vector.tensor_scalar(
                            out=mw[:], in0=relb[:],
                            scalar1=cst[:, blk:blk + 1],
                            scalar2=gwa[:, 4 * t + blk:4 * t + blk + 1],
                            op0=OP.is_equal, op1=OP.mult)
                        nc.tensor.matmul(out=ps_s[:, :], lhsT=bdiag8[:],
                                         rhs=mw[:], start=(blk == 0),
                                         stop=False,
                                         skip_group_check=(blk > 0))
                    if t not in TH_SET:
                        # dst via Pool 16-candidate route
                        i = pool_pos[t]
                        cand = cndp.tile([128, 512], f32, tag="cand")
                        nc.gpsimd.indirect_copy(
                            out=cand[:], data=candtab[:],
                            idxs=icidx_all[:, i],
                            i_know_ap_gather_is_preferred=True)
                        ecf8 = gatep.tile([8, 512], bf16, tag="ecf8")
                        nc.sync.dma_start(out=ecf8[:], in_=ecf_d[i])
                        psD = bcp.tile([128, 512], f32, tag="bc")
                        nc.tensor.matmul(
                            out=psD[:], lhsT=expand8[:],
                            rhs=ecf8[:], start=True, stop=True)
                        msk = mkp.tile([128, 512], bf16, tag="msk")
                        nc.vector.scalar_tensor_tensor(
                            out=msk[:], in0=psD[:], scalar=cst[:, 0:1],
                            in1=cand[:], op0=OP.is_equal, op1=OP.mult)
                        nc.tensor.matmul(out=ps_s[:, :], lhsT=bdiag8[:],
                                         rhs=msk[:], start=False, stop=False,
                                         skip_group_check=True)
                    else:
                        i = th_pos[t]
                        def th_bcast(u):
                            psL = bcp.tile([128, 512], f32, tag="bc")
                            nc.tensor.matmul(
                                out=psL[:], lhsT=usel[u][:],
                                rhs=lo_all[:, i], start=True, stop=True)
                            psH = bhp.tile([128, 512], f32, tag="bh")
                            nc.tensor.matmul(
                                out=psH[:], lhsT=usel[u][:],
                                rhs=hi_all[:, i], start=True, stop=True)
                            lob = mkp.tile([128, 512], f16, tag="lob")
                            nc.scalar.activation(out=lob[:], in_=psL[:],
                                                 func=ACT.Identity)
                            return psH, lob
                        nxt = th_bcast(0)
                        for u in range(8):
                            psH, lob = nxt
                            if u < 7:
                                nxt = th_bcast(u + 1)
                            Hm = mkp.tile([128, 512], f16, tag="Hm")
                            nc.vector.tensor_scalar(
                                out=Hm[:], in0=psH[:], scalar1=cst[:, 3:4],
                                scalar2=None, op0=OP.is_equal)
                            psS2 = s2p.tile([128, 512], f32, tag="psS2")
                            for k in range(4):
                                Mk = mkp.tile([128, 512], f16,
                                              name=f"Mk{k}", tag=f"Mk{k}")
                                nc.vector.tensor_scalar(
                                    out=Mk[:], in0=lob[:],
                                    scalar1=cst[:, 3 + k:4 + k],
                                    scalar2=None, op0=OP.is_equal)
                                nc.tensor.matmul(out=psS2[:], lhsT=TT[k][:],
                                                 rhs=Mk[:], start=(k == 0),
                                                 stop=(k == 3))
                            masked = mkp.tile([128, 512], f16, tag="maskd")
                            nc.vector.scalar_tensor_tensor(
                                out=masked[:], in0=psS2[:], scalar=0.0,
                                in1=Hm[:], op0=OP.add, op1=OP.mult)
                            nc.tensor.matmul(out=ps_s[:, :],
                                             lhsT=osel[u][:], rhs=masked[:],
                                             start=False, stop=False,
                                             skip_group_check=True)
                    # gate: one Ln over [16,512] (rows 0-7 a1-affine,
                    # rows 8-15 a2-affine), then row-half subtract
                    nz = gatep.tile([16, 512], f32, tag="nz")
                    nc.sync.dma_start(out=nz[0:8, :], in_=nz_d[t])
                    nc.sync.dma_start(out=nz[8:16, :], in_=nz_d[t])
                    t12 = gatep.tile([16, 512], f32, tag="t12")
                    nc.scalar.activation(out=t12[:], in_=nz[:], func=ACT.Ln,
                                         bias=gc[:, 1:2], scale=gc[:, 0:1])
                    # t1 - t2 accumulated straight into ps_s via +/-1 selector
                    nc.tensor.matmul(out=ps_s[:, :], lhsT=tsel[:],
                                     rhs=t12[:], start=False, stop=True,
                                     skip_group_check=True)
                    gt2 = g2p.tile([8, 512], f32, tag="gt2")
                    nc.scalar.activation(out=gt2[:], in_=ps_s[:],
                                         func=ACT.Identity, bias=bE[:, 0:1])
                    pend.append((t, gt2))
                    if len(pend) == GGRP or t == NTIL - 1:
                        for tt, g2 in pend:
                            ot = gatep.tile([8, 512], f32, tag="ot")
                            nc.scalar.activation(out=ot[:], in_=g2[:],
                                                 func=ACT.Sigmoid)
                            nc.sync.dma_start(out=out_d[tt], in_=ot[:])
                        pend = []
    nc.compile()
    return nc


def _get_nc():
    global _nc
    if _nc is None:
        _nc = _build()
    return _nc


def prepare_in_maps(embedding, edges, noise, W_emb, b_emb, W_edge, b_edge):
    import ml_dtypes
    bf = ml_dtypes.bfloat16
    embedding = np.asarray(embedding, dtype=np.float32)
    edges = np.asarray(edges)
    noise = np.asarray(noise, dtype=np.float32)
    W_emb = np.asarray(W_emb, dtype=np.float32)
    b_emb = np.asarray(b_emb, dtype=np.float32)
    W_edge = np.asarray(W_edge, dtype=np.float32)
    b_edge = np.float32(b_edge)

    wbar = ((W_edge[:HID] + W_edge[HID:]) * 0.5).astype(np.float32)
    wemb = np.ascontiguousarray(W_emb.reshape(2, 128, HID))
    bemb = np.ascontiguousarray(b_emb.reshape(HID, 1))
    wbarr = np.ascontiguousarray(wbar.reshape(HID, 1))
    bE = np.full((8, 1), b_edge, dtype=np.float32)
    a1, b1 = 2.0 * BIAS - 1.0, 1.0 - BIAS
    a2, b2 = 1.0 - 2.0 * BIAS, BIAS
    gc = np.zeros((16, 2), dtype=np.float32)
    gc[0:8, 0], gc[0:8, 1] = a1, b1
    gc[8:16, 0], gc[8:16, 1] = a2, b2
    tsel = np.zeros((16, 8), dtype=np.float32)
    for r in range(8):
        tsel[r, r] = 1.0
        tsel[8 + r, r] = -1.0
    p = np.arange(128)
    cst = np.zeros((128, 7), dtype=np.float32)
    q = p % 16
    cst[:, 0] = q
    cst[:, 1] = q + 16
    cst[:, 2] = q + 32
    cst[:, 3] = p
    cst[:, 4] = 128 + p
    cst[:, 5] = 256 + p
    cst[:, 6] = np.where(p < C2 - 384, 384 + p, 1e9)
    ones1 = np.ones((1, 128), dtype=bf)
    ones128 = np.ones((128, 1), dtype=bf)
    expand8 = (p[None, :] // 16 == np.arange(8)[:, None]).astype(bf)
    bdiag8 = (p[:, None] // 16 == np.arange(8)[None, :]).astype(bf)
    osel = np.zeros((8, 128, 8), dtype=np.float16)
    for u in range(8):
        osel[u, :, u] = 1.0
    usel = np.zeros((8, 8, 128), dtype=np.float16)
    for u in range(8):
        usel[u, u, :] = 1.0

    e0 = edges[0].astype(np.int64)
    e1 = edges[1].astype(np.int64)
    ord0 = np.argsort(e0, kind="stable")
    e0s = e0[ord0].astype(np.int32)
    e1s = e1[ord0].astype(np.int32)
    nzs = noise[ord0]

    in_maps = []
    orig_ids = np.empty((NCORES, ECP), dtype=np.int64)
    for k in range(NCORES):
        sl = slice(k * EC, (k + 1) * EC)
        e0c = np.concatenate([e0s[sl], np.full(ECP - EC, e0s[sl][-1],
                                               dtype=np.int32)])
        e1c = np.concatenate([e1s[sl], np.zeros(ECP - EC, dtype=np.int32)])
        nzc = np.concatenate([nzs[sl], np.full(ECP - EC, 0.5,
                                               dtype=np.float32)])
        oc = np.concatenate([ord0[sl], np.full(ECP - EC, -1, dtype=np.int64)])
        orig_ids[k] = oc
        e0b = e0c.reshape(NTIL, 8, 512)
        bg16 = e0b[:, :, 0] // 16                  # [NTIL, 8]
        rel = e0b - (bg16 * 16)[:, :, None]
        assert rel.min() >= 0 and rel.max() < 48, (rel.min(), rel.max())
        rel_in = rel.astype(bf)
        nz_in = nzc.reshape(NTIL, 8, 512)
        # window-fetch idxs: [16g+blk, t] = bg16[t, g] + blk (blk < 3)
        gwidx = np.zeros((128, 16), dtype=np.uint16)
        for g in range(8):
            for j in range(208):
                t_, blk = j // 4, j % 4
                v = bg16[t_, g] + blk if t_ < NTIL else 0
                gwidx[16 * g + j % 16, j // 16] = v
        # pool tiles
        e1t = e1c.reshape(NTIL, TILE)
        pool_ord = [t for t in range(NTIL) if t not in TH_SET]
        icidx = np.zeros((max(NPOOL, 1), 128, 32), dtype=np.uint16)
        ecf = np.zeros((max(NPOOL, 1), 8, 512), dtype=bf)
        for i, t in enumerate(pool_ord):
            v = e1t[t].reshape(8, 512)
            ecf[i] = (v & 15).astype(bf)
            w = (v >> 4).astype(np.uint16).reshape(8, 32, 16)
            icidx[i] = w.transpose(0, 2, 1).reshape(128, 32)
        lo_in = np.zeros((max(NTH, 1), 8, 512), dtype=np.float16)
        hi_in = np.zeros((max(NTH, 1), 8, 512), dtype=np.float16)
        for i, t in enumerate(TH_SET):
            v = e1t[t].reshape(8, 512)
            hi = v // C2
            lo_in[i] = (v - hi * C2).astype(np.float16)
            hi_in[i] = hi.astype(np.float16)
        sl_emb = embedding[k * NNC:(k + 1) * NNC]
        embT = np.zeros((IN_DIM, NNCP), dtype=np.float32)
        embT[:, :NNC] = sl_emb.T
        in_maps.append({
            "embT": np.ascontiguousarray(embT.reshape(2, 128, NNCP)),
            "wemb": wemb, "bemb": bemb, "wbar": wbarr, "bE": bE,
            "cst": cst, "ones1": ones1, "ones128": ones128,
            "expand8": expand8, "bdiag8": bdiag8,
            "osel": osel, "usel": usel, "gc": gc, "tsel": tsel,
            "rel": rel_in, "nz": nz_in, "gwidx": gwidx,
            "icidx": icidx, "ecf": ecf, "lo": lo_in, "hi": hi_in,
        })
    return in_maps, orig_ids


def kernel(embedding, edges, noise, W_emb, b_emb, W_edge, b_edge):
    from concourse import bass_utils
    nc = _get_nc()
    in_maps, orig_ids = prepare_in_maps(embedding, edges, noise, W_emb,
                                        b_emb, W_edge, b_edge)
    res = bass_utils.run_bass_kernel_spmd(nc, in_maps,
                                          core_ids=list(range(NCORES)))
    out = np.empty(E, dtype=np.float32)
    for k in range(NCORES):
        o = res.results[k]["out"].reshape(ECP)
        ids = orig_ids[k]
        m = ids >= 0
        out[ids[m]] = o[m]
    return out


# revision 4
# speedup vs baseline: 1.1833x; 1.1833x over previous
"""Sorted-window + two-hot gather kernel.

Host globally sorts edges by src and shards contiguous ranges to cores.
Per 4096-edge tile the src values span <256 nodes, so the src gather is a
windowed one-hot (PE broadcast -> ACT copy -> 2 DVE compares -> 2 PE
accumulating matmuls). The dst side is random over all 50K nodes and is
split between the Pool 16-candidate indirect_copy route (NPOOL tiles) and
a factorized "two-hot" gather (NTH tiles): dst = hi*391 + lo, one-hot of
lo via 4 DVE compares feeding 4 PE matmuls against [128,128] slices of the
g-table, then a hi partition mask and a ones-reduce. All routes accumulate
s = g[src]+g[dst] in one PSUM tile; ACT computes the gumbel-sigmoid gate.
Output is returned in sorted order and unpermuted on the host.
"""
import sys
sys.path.insert(0, '/opt/trn_rl_repo')
import numpy as np

N, IN_DIM, HID = 50000, 256, 64
E = 1_600_000
BIAS = 0.0001
NCORES = 8
EC = E // NCORES            # 200000
TILE = 4096
NTIL = 49
ECP = NTIL * TILE           # 200704
NNC = N // NCORES           # 6250
NNCP = 6272
C2 = 391                    # two-hot stride: dst = hi*391 + lo, hi < 128
GPAD = 50432                # padded g table: 384 + 128*391 = 50432
TABW = GPAD // 16           # 3152
NTH = 20                    # two-hot dst tiles
NPOOL = NTIL - NTH          # 35 pool dst tiles
TH_SET = sorted({int((i + 0.5) * NTIL / NTH) for i in range(NTH)})
assert len(TH_SET) == NTH

_nc = None
NREP = 1
SIMM = False


def _build():
    from concourse import bass, bacc, tile, mybir

    f32 = mybir.dt.float32
    bf16 = mybir.dt.bfloat16
    u16 = mybir.dt.uint16
    f16 = mybir.dt.float16
    ACT = mybir.ActivationFunctionType
    OP = mybir.AluOpType
    nc = bacc.Bacc("TRN2", target_bir_lowering=False, debug=False,
                   num_devices=(1 if SIMM else NCORES))

    embT_d = nc.dram_tensor("embT", [2, 128, NNCP], f32, kind="ExternalInput")
    wemb_d = nc.dram_tensor("wemb", [2, 128, HID], f32, kind="ExternalInput")
    bemb_d = nc.dram_tensor("bemb", [HID, 1], f32, kind="ExternalInput")
    wbar_d = nc.dram_tensor("wbar", [HID, 1], f32, kind="ExternalInput")
    bE_d = nc.dram_tensor("bE", [8, 1], f32, kind="ExternalInput")
    gc_d = nc.dram_tensor("gc", [16, 2], f32, kind="ExternalInput")
    tsel_d = nc.dram_tensor("tsel", [16, 8], f32, kind="ExternalInput")
    cst_d = nc.dram_tensor("cst", [128, 7], f32, kind="ExternalInput")
    ones1_d = nc.dram_tensor("ones1", [1, 128], bf16, kind="ExternalInput")
    ones128_d = nc.dram_tensor("ones128", [128, 1], bf16, kind="ExternalInput")
    expand8_d = nc.dram_tensor("expand8", [8, 128], bf16, kind="ExternalInput")
    bdiag8_d = nc.dram_tensor("bdiag8", [128, 8], bf16, kind="ExternalInput")
    osel_d = nc.dram_tensor("osel", [8, 128, 8], f16, kind="ExternalInput")
    usel_d = nc.dram_tensor("usel", [8, 8, 128], f16, kind="ExternalInput")
    rel_d = nc.dram_tensor("rel", [NTIL, 8, 512], bf16, kind="ExternalInput")
    nz_d = nc.dram_tensor("nz", [NTIL, 8, 512], f32, kind="ExternalInput")
    gwidx_d = nc.dram_tensor("gwidx", [128, 16], u16, kind="ExternalInput")
    icidx_d = nc.dram_tensor("icidx", [max(NPOOL, 1), 128, 32], u16,
                             kind="ExternalInput")
    ecf_d = nc.dram_tensor("ecf", [max(NPOOL, 1), 8, 512], bf16,
                           kind="ExternalInput")
    lo_d = nc.dram_tensor("lo", [max(NTH, 1), 8, 512], f16,
                          kind="ExternalInput")
    hi_d = nc.dram_tensor("hi", [max(NTH, 1), 8, 512], f16,
                          kind="ExternalInput")
    out_d = nc.dram_tensor("out", [NTIL, 8, 512], f32, kind="ExternalOutput")

    a1, b1 = 2.0 * BIAS - 1.0, 1.0 - BIAS
    a2, b2 = 1.0 - 2.0 * BIAS, BIAS

    with tile.TileContext(nc) as tc:
        with tc.tile_pool(name="const", bufs=1) as cp, \
             tc.tile_pool(name="tab", bufs=1) as tabp, \
             tc.tile_pool(name="dram", bufs=1, space="DRAM") as dram:
            def cload(name, shape, dt, src):
                t = cp.tile(shape, dt, name=name, tag=name)
                nc.sync.dma_start(out=t[:], in_=src)
                return t
            w0 = cload("w0", [128, HID], f32, wemb_d[0])
            w1 = cload("w1", [128, HID], f32, wemb_d[1])
            bemb = cload("bemb", [HID, 1], f32, bemb_d[:, :])
            wbar = cload("wbar", [HID, 1], f32, wbar_d[:, :])
            bE = cload("bE", [8, 1], f32, bE_d[:, :])
            cst = cload("cst", [128, 7], f32, cst_d[:, :])
            ones1 = cload("ones1", [1, 128], bf16, ones1_d[:, :])
            ones128 = cload("ones128", [128, 1], bf16, ones128_d[:, :])
            expand8 = cload("expand8", [8, 128], bf16, expand8_d[:, :])
            bdiag8 = cload("bdiag8", [128, 8], bf16, bdiag8_d[:, :])
            osel = [cload(f"osel{u}", [128, 8], f16, osel_d[u])
                    for u in range(8)]
            usel = [cload(f"usel{u}", [8, 128], f16, usel_d[u])
                    for u in range(8)]
            gc = cload("gc", [16, 2], f32, gc_d[:, :])
            tsel = cload("tsel", [16, 8], f32, tsel_d[:, :])
            # bulk per-tile index/mask inputs, u-major in SBUF
            lo_all = cp.tile([8, max(NTH, 1), 512], f16, tag="lo_all")
            nc.sync.dma_start(out=lo_all[:],
                              in_=lo_d[:, :, :].rearrange("t u m -> u t m"))
            hi_all = cp.tile([8, max(NTH, 1), 512], f16, tag="hi_all")
            nc.sync.dma_start(out=hi_all[:],
                              in_=hi_d[:, :, :].rearrange("t u m -> u t m"))
            icidx_all = cp.tile([128, max(NPOOL, 1), 32], u16, tag="icidx_all")
            nc.sync.dma_start(out=icidx_all[:],
                              in_=icidx_d[:, :, :].rearrange("t p w -> p t w"))
            gwidx = cp.tile([128, 16], u16, tag="gwidx")
            nc.sync.dma_start(out=gwidx[:], in_=gwidx_d[:, :])

            # ---------- phase 1: per-node scalar g ----------
            g_sb = cp.tile([1, NNCP], f32, tag="gsb")
            with tc.tile_pool(name="p1", bufs=3) as p1, \
                 tc.tile_pool(name="ps1", bufs=2, space="PSUM") as ps1, \
                 tc.tile_pool(name="ps1g", bufs=2, space="PSUM") as ps1g:
                col = 0
                while col < NNCP:
                    n = min(512, NNCP - col)
                    r0 = p1.tile([128, n], f32, tag="r0")
                    r1 = p1.tile([128, n], f32, tag="r1")
                    nc.sync.dma_start(out=r0[:], in_=embT_d[0, :, col:col + n])
                    nc.sync.dma_start(out=r1[:], in_=embT_d[1, :, col:col + n])
                    ph = ps1.tile([HID, n], f32, tag="ph")
                    nc.tensor.matmul(out=ph[:], lhsT=w0[:], rhs=r0[:],
                                     start=True, stop=False)
                    nc.tensor.matmul(out=ph[:], lhsT=w1[:], rhs=r1[:],
                                     start=False, stop=True)
                    hT = p1.tile([HID, n], f32, tag="hT")
                    nc.scalar.activation(out=hT[:], in_=ph[:], func=ACT.Relu,
                                         bias=bemb[:, 0:1])
                    pg = ps1g.tile([1, n], f32, tag="pg")
                    nc.tensor.matmul(out=pg[:], lhsT=wbar[:], rhs=hT[:],
                                     start=True, stop=True)
                    nc.scalar.activation(out=g_sb[0:1, col:col + n], in_=pg[:],
                                         func=ACT.Identity)
                    col += n

            g_mine = dram.tile([1, NNC], f32, tag="gmine")
            g_all = dram.tile([1, GPAD], f32, tag="gall")
            nc.sync.dma_start(out=g_mine[:], in_=g_sb[0:1, 0:NNC])
            zpad = cp.tile([1, GPAD - N], f32, tag="zpad")
            nc.vector.memset(zpad[:], 0.0)
            nc.sync.dma_start(out=g_all[0:1, N:GPAD], in_=zpad[:])
            if SIMM:
                nc.sync.dma_start(out=g_all[0:1, 0:NNC], in_=g_mine[:])
            else:
                nc.gpsimd.collective_compute(
                    "AllGather", bass.mybir.AluOpType.bypass,
                    replica_groups=[list(range(NCORES))],
                    ins=[g_mine[:].opt()], outs=[g_all[0:1, 0:N].opt()])

            # ---------- tables ----------
            candtab = tabp.tile([128, TABW], f32, tag="candtab")
            g_wc = g_all[0, 0:GPAD].rearrange("(w c) -> c w", c=16)
            dma_engs = [nc.sync, nc.scalar]
            for G in range(8):
                dma_engs[G % 2].dma_start(
                    out=candtab[16 * G:16 * G + 16, :], in_=g_wc)
            TT = []
            for k in range(4):
                tmp = tabp.tile([128, 128], f32, name=f"ttf{k}", tag=f"ttf{k}")
                src_ap = g_all[0, 128 * k:128 * k + 128 * C2].rearrange(
                    "(i c) -> c i", c=C2)
                nc.sync.dma_start(out=tmp[:], in_=src_ap[0:128, :])
                ttk = tabp.tile([128, 128], f16, name=f"tt{k}", tag=f"tt{k}")
                nc.vector.tensor_copy(out=ttk[:], in_=tmp[:])
                TT.append(ttk)

            # batched per-band window fetch for all tiles:
            # gwa[16g+q, 4t+blk] = g[16*bg16[t,g] + 16*blk + q]
            gwa = tabp.tile([128, 208], f32, tag="gwa")
            nc.gpsimd.indirect_copy(
                out=gwa[:], data=candtab[:], idxs=gwidx[:, :],
                i_know_ap_gather_is_preferred=True)

            # ---------- phase 2 ----------
            pool_ord = [t for t in range(NTIL) if t not in TH_SET]
            th_ord = TH_SET
            pool_pos = {t: i for i, t in enumerate(pool_ord)}
            th_pos = {t: i for i, t in enumerate(th_ord)}

            import contextlib
            rep_ctx = tc.For_i(0, NREP) if NREP > 1 else contextlib.nullcontext()
            with rep_ctx, \
                 tc.tile_pool(name="gw", bufs=3) as gwp, \
                 tc.tile_pool(name="bc", bufs=2, space="PSUM") as bcp, \
                 tc.tile_pool(name="s2", bufs=2, space="PSUM") as s2p, \
                 tc.tile_pool(name="bh", bufs=2, space="PSUM") as bhp, \
                 tc.tile_pool(name="pss", bufs=2, space="PSUM") as pssp, \
                 tc.tile_pool(name="mk", bufs=3) as mkp, \
                 tc.tile_pool(name="cnd", bufs=2) as cndp, \
                 tc.tile_pool(name="gate", bufs=2) as gatep, \
                 tc.tile_pool(name="g2", bufs=4) as g2p:
                pend = []
                GGRP = 3
                for t in range(NTIL):
                    ps_s = pssp.tile([8, 512], f32, tag="ps_s")
                    # src: band-windowed one-hot, whole tile at once
                    rel8 = gatep.tile([8, 512], bf16, tag="rel8")
                    nc.sync.dma_start(out=rel8[:], in_=rel_d[t])
                    psB = bcp.tile([128, 512], f32, tag="bc")
                    nc.tensor.matmul(out=psB[:], lhsT=expand8[:],
                                     rhs=rel8[:], start=True, stop=True)
                    relb = mkp.tile([128, 512], bf16, tag="relb")
                    nc.scalar.activation(out=relb[:], in_=psB[:],
                                         func=ACT.Identity)
                    for blk in range(3):
                        mw = mkp.tile([128, 512], bf16, name=f"mw{blk}",
                                      tag=f"mw{blk}")
                        nc.vector.tensor_scalar(
                            out=mw[:], in0=relb[:],
                            scalar1=cst[:, blk:blk + 1],
                            scalar2=gwa[:, 4 * t + blk:4 * t + blk + 1],
                            op0=OP.is_equal, op1=OP.mult)
                        nc.tensor.matmul(out=ps_s[:, :], lhsT=bdiag8[:],
                                         rhs=mw[:], start=(blk == 0),
                                         stop=False,
                                         skip_group_check=(blk > 0))
                    if t not in TH_SET:
                        # dst via Pool 16-candidate route
                        i = pool_pos[t]
                        cand = cndp.tile([128, 512], f32, tag="cand")
                        nc.gpsimd.indirect_copy(
                            out=cand[:], data=candtab[:],
                            idxs=icidx_all[:, i],
                            i_know_ap_gather_is_preferred=True)
                        ecf8 = gatep.tile([8, 512], bf16, tag="ecf8")
                        nc.sync.dma_start(out=ecf8[:], in_=ecf_d[i])
                        psD = bcp.tile([128, 512], f32, tag="bc")
                        nc.tensor.matmul(
                            out=psD[:], lhsT=expand8[:],
                            rhs=ecf8[:], start=True, stop=True)
                        msk = mkp.tile([128, 512], bf16, tag="msk")
                        nc.vector.scalar_tensor_tensor(
                            out=msk[:], in0=psD[:], scalar=cst[:, 0:1],
                            in1=cand[:], op0=OP.is_equal, op1=OP.mult)
                        nc.tensor.matmul(out=ps_s[:, :], lhsT=bdiag8[:],
                                         rhs=msk[:], start=False, stop=False,
                                         skip_group_check=True)
                    else:
                        i = th_pos[t]
                        def th_bcast(u):
                            psL = bcp.tile([128, 512], f32, tag="bc")
                            nc.tensor.matmul(
                                out=psL[:], lhsT=usel[u][:],
                                rhs=lo_all[:, i], start=True, stop=True)
                            psH = bhp.tile([128, 512], f32, tag="bh")
                            nc.tensor.matmul(
                                out=psH[:], lhsT=usel[u][:],
                                rhs=hi_all[:, i], start=True, stop=True)
                            lob = mkp.tile([128, 512], f16, tag="lob")
                            nc.scalar.activation(out=lob[:], in_=psL[:],
                                                 func=ACT.Identity)
                            return psH, lob
                        nxt = th_bcast(0)
                        for u in range(8):
                            psH, lob = nxt
                            if u < 7:
                                nxt = th_bcast(u + 1)
                            Hm = mkp.tile([128, 512], f16, tag="Hm")
                            nc.vector.tensor_scalar(
                                out=Hm[:], in0=psH[:], scalar1=cst[:, 3:4],
                                scalar2=None, op0=OP.is_equal)
                            psS2 = s2p.tile([128, 512], f32, tag="psS2")
                            for k in range(4):
                                Mk = mkp.tile([128, 512], f16,
                                              name=f"Mk{k}", tag=f"Mk{k}")
                                nc.vector.tensor_scalar(
                                    out=Mk[:], in0=lob[:],
                                    scalar1=cst[:, 3 + k:4 + k],
                                    scalar2=None, op0=OP.is_equal)
                                nc.tensor.matmul(out=psS2[:], lhsT=TT[k][:],
                                                 rhs=Mk[:], start=(k == 0),
                                                 stop=(k == 3))
                            masked = mkp.tile([128, 512], f16, tag="maskd")
                            nc.vector.scalar_tensor_tensor(
                                out=masked[:], in0=psS2[:], scalar=0.0,
                                in1=Hm[:], op0=OP.add, op1=OP.mult)
                            nc.tensor.matmul(out=ps_s[:, :],
                                             lhsT=osel[u][:], rhs=masked[:],
                                             start=False, stop=False,
                                             skip_group_check=True)
                    # gate: one Ln over [16,512] (rows 0-7 a1-affine,
                    # rows 8-15 a2-affine), then row-half subtract
                    nz = gatep.tile([16, 512], f32, tag="nz")
                    nc.sync.dma_start(out=nz[0:8, :], in_=nz_d[t])
                    nc.sync.dma_start(out=nz[8:16, :], in_=nz_d[t])
                    t12 = gatep.tile([16, 512], f32, tag="t12")
                    nc.scalar.activation(out=t12[:], in_=nz[:], func=ACT.Ln,
                                         bias=gc[:, 1:2], scale=gc[:, 0:1])
                    # t1 - t2 accumulated straight into ps_s via +/-1 selector
                    nc.tensor.matmul(out=ps_s[:, :], lhsT=tsel[:],
                                     rhs=t12[:], start=False, stop=True,
                                     skip_group_check=True)
                    gt2 = g2p.tile([8, 512], f32, tag="gt2")
                    nc.scalar.activation(out=gt2[:], in_=ps_s[:],
                                         func=ACT.Identity, bias=bE[:, 0:1])
                    pend.append((t, gt2))
                    if len(pend) == GGRP or t == NTIL - 1:
                        for tt, g2 in pend:
                            ot = gatep.tile([8, 512], f32, tag="ot")
                            nc.scalar.activation(out=ot[:], in_=g2[:],
                                                 func=ACT.Sigmoid)
                            nc.sync.dma_start(out=out_d[tt], in_=ot[:])
                        pend = []
    nc.compile()
    return nc


def _get_nc():
    global _nc
    if _nc is None:
        _nc = _build()
    return _nc


def prepare_in_maps(embedding, edges, noise, W_emb, b_emb, W_edge, b_edge):
    import ml_dtypes
    bf = ml_dtypes.bfloat16
    embedding = np.asarray(embedding, dtype=np.float32)
    edges = np.asarray(edges)
    noise = np.asarray(noise, dtype=np.float32)
    W_emb = np.asarray(W_emb, dtype=np.float32)
    b_emb = np.asarray(b_emb, dtype=np.float32)
    W_edge = np.asarray(W_edge, dtype=np.float32)
    b_edge = np.float32(b_edge)

    wbar = ((W_edge[:HID] + W_edge[HID:]) * 0.5).astype(np.float32)
    wemb = np.ascontiguousarray(W_emb.reshape(2, 128, HID))
    bemb = np.ascontiguousarray(b_emb.reshape(HID, 1))
    wbarr = np.ascontiguousarray(wbar.reshape(HID, 1))
    bE = np.full((8, 1), b_edge, dtype=np.float32)
    a1, b1 = 2.0 * BIAS - 1.0, 1.0 - BIAS
    a2, b2 = 1.0 - 2.0 * BIAS, BIAS
    gc = np.zeros((16, 2), dtype=np.float32)
    gc[0:8, 0], gc[0:8, 1] = a1, b1
    gc[8:16, 0], gc[8:16, 1] = a2, b2
    tsel = np.zeros((16, 8), dtype=np.float32)
    for r in range(8):
        tsel[r, r] = 1.0
        tsel[8 + r, r] = -1.0
    p = np.arange(128)
    cst = np.zeros((128, 7), dtype=np.float32)
    q = p % 16
    cst[:, 0] = q
    cst[:, 1] = q + 16
    cst[:, 2] = q + 32
    cst[:, 3] = p
    cst[:, 4] = 128 + p
    cst[:, 5] = 256 + p
    cst[:, 6] = np.where(p < C2 - 384, 384 + p, 1e9)
    ones1 = np.ones((1, 128), dtype=bf)
    ones128 = np.ones((128, 1), dtype=bf)
    expand8 = (p[None, :] // 16 == np.arange(8)[:, None]).astype(bf)
    bdiag8 = (p[:, None] // 16 == np.arange(8)[None, :]).astype(bf)
    osel = np.zeros((8, 128, 8), dtype=np.float16)
    for u in range(8):
        osel[u, :, u] = 1.0
    usel = np.zeros((8, 8, 128), dtype=np.float16)
    for u in range(8):
        usel[u, u, :] = 1.0

    e0 = edges[0].astype(np.int64)
    e1 = edges[1].astype(np.int64)
    ord0 = np.argsort(e0, kind="stable")
    e0s = e0[ord0].astype(np.int32)
    e1s = e1[ord0].astype(np.int32)
    nzs = noise[ord0]

    in_maps = []
    orig_ids = np.empty((NCORES, ECP), dtype=np.int64)
    for k in range(NCORES):
        sl = slice(k * EC, (k + 1) * EC)
        e0c = np.concatenate([e0s[sl], np.full(ECP - EC, e0s[sl][-1],
                                               dtype=np.int32)])
        e1c = np.concatenate([e1s[sl], np.zeros(ECP - EC, dtype=np.int32)])
        nzc = np.concatenate([nzs[sl], np.full(ECP - EC, 0.5,
                                               dtype=np.float32)])
        oc = np.concatenate([ord0[sl], np.full(ECP - EC, -1, dtype=np.int64)])
        orig_ids[k] = oc
        e0b = e0c.reshape(NTIL, 8, 512)
        bg16 = e0b[:, :, 0] // 16                  # [NTIL, 8]
        rel = e0b - (bg16 * 16)[:, :, None]
        assert rel.min() >= 0 and rel.max() < 48, (rel.min(), rel.max())
        rel_in = rel.astype(bf)
        nz_in = nzc.reshape(NTIL, 8, 512)
        # window-fetch idxs: [16g+blk, t] = bg16[t, g] + blk (blk < 3)
        gwidx = np.zeros((128, 16), dtype=np.uint16)
        for g in range(8):
            for j in range(208):
                t_, blk = j // 4, j % 4
                v = bg16[t_, g] + blk if t_ < NTIL else 0
                gwidx[16 * g + j % 16, j // 16] = v
        # pool tiles
        e1t = e1c.reshape(NTIL, TILE)
        pool_ord = [t for t in range(NTIL) if t not in TH_SET]
        icidx = np.zeros((max(NPOOL, 1), 128, 32), dtype=np.uint16)
        ecf = np.zeros((max(NPOOL, 1), 8, 512), dtype=bf)
        for i, t in enumerate(pool_ord):
            v = e1t[t].reshape(8, 512)
            ecf[i] = (v & 15).astype(bf)
            w = (v >> 4).astype(np.uint16).reshape(8, 32, 16)
            icidx[i] = w.transpose(0, 2, 1).reshape(128, 32)
        lo_in = np.zeros((max(NTH, 1), 8, 512), dtype=np.float16)
        hi_in = np.zeros((max(NTH, 1), 8, 512), dtype=np.float16)
        for i, t in enumerate(TH_SET):
            v = e1t[t].reshape(8, 512)
            hi = v // C2
            lo_in[i] = (v - hi * C2).astype(np.float16)
            hi_in[i] = hi.astype(np.float16)
        sl_emb = embedding[k * NNC:(k + 1) * NNC]
        embT = np.zeros((IN_DIM, NNCP), dtype=np.float32)
        embT[:, :NNC] = sl_emb.T
        in_maps.append({
            "embT": np.ascontiguousarray(embT.reshape(2, 128, NNCP)),
            "wemb": wemb, "bemb": bemb, "wbar": wbarr, "bE": bE,
            "cst": cst, "ones1": ones1, "ones128": ones128,
            "expand8": expand8, "bdiag8": bdiag8,
            "osel": osel, "usel": usel, "gc": gc, "tsel": tsel,
            "rel": rel_in, "nz": nz_in, "gwidx": gwidx,
            "icidx": icidx, "ecf": ecf, "lo": lo_in, "hi": hi_in,
        })
    return in_maps, orig_ids


def kernel(embedding, edges, noise, W_emb, b_emb, W_edge, b_edge):
    from concourse import bass_utils
    nc = _get_nc()
    in_maps, orig_ids = prepare_in_maps(embedding, edges, noise, W_emb,
                                        b_emb, W_edge, b_edge)
    res = bass_utils.run_bass_kernel_spmd(nc, in_maps,
                                          core_ids=list(range(NCORES)))
    out = np.empty(E, dtype=np.float32)
    for k in range(NCORES):
        o = res.results[k]["out"].reshape(ECP)
        ids = orig_ids[k]
        m = ids >= 0
        out[ids[m]] = o[m]
    return out
